# revision 23
# baseline (speedup 1.0000x reference)
"""2x2/stride-2 max-pool (NCHW, padding=0) on Trainium2, data-parallel over 8 cores.

Problem: x (32, 96, 224, 224) fp32 -> out (32, 96, 112, 112) fp32.

Sharding: pure data parallel on the batch dim — core i handles x[4i:4i+4].

The kernel is pure HBM-bandwidth (target_regime=memory): every input element
is read once and every output written once.  Two levers:

1. Precision.  The accuracy gate is 2e-2 relative error; a round-to-nearest
   fp32->bf16 cast of the inputs changes the pooled max by at most 2^-9
   (~2e-3) relative — max(rn(x)) == rn(max(x)) since rounding is monotone.
   The host casts x to bf16, the device pools in bf16, the host widens the
   result back to fp32.  Halves HBM traffic: 96.3 MB -> 48.2 MB per core.
   (Measured rel err 3e-3.  Reads then cap at the ~354 GB/s HBM-per-core
   rate; stores ride concurrently on the same 16 SDMA engines.)

2. DVE perf mode.  A stride-2 operand (adjacent-column max) runs the DVE at
   1 elem/cycle; packed stride-1 bf16 operands run at 2.  So the host
   permutes each (2x224)-row-pair into [r0_even | r1_even | r0_odd | r1_odd]
   (a pure within-row byte permutation — DMA still fully contiguous) and the
   device reduces with two packed tensor_max halvings: halves(448) ->
   column-max pair (224) -> halves(224) -> pooled row (112).  Both ops hit
   the 2x mode: 7.3 us per 42-row chunk instead of 10.1.

Per core the (4, 96, 224, 224) shard is 43008 row-pairs of 448 contiguous
bf16.  Each chunk loads a fully contiguous [128 x Mc row-pairs] block,
reduces it, and stores a fully contiguous [128 x Mc*112] block.  Loads ride
the SP HWDGE ring, stores the ACT ring (dedicating each ring to one
direction measured faster than alternating).  The final chunks descend
(10/7/5/3/3) to minimize the end-of-kernel load->max->max->store chain
(see TAIL comment), and the last store rides the idle SP ring.
"""

import os

import ml_dtypes
import numpy as np

N_CORES = 8
PAIRS = 43008               # row-pairs per core: 4*96*224/2
MAINS = [44] * 6 + [22, 22]  # main chunks (SP-ring loads).  The last 44 is
                            # split into two 22s: the end-of-kernel chain is
                            # bound by (last main's load end + its compute +
                            # tail computes), and halving the last main lets
                            # its first half's compute start ~7 us earlier —
                            # worth ~2.5 us even though 22-row tensor_max
                            # runs below the packed-2x rate.  NOTE: DVE
                            # tensor_max hits its full 2x rate (~1.93 el/ns)
                            # at 40/42/44-row chunks but drops to ~1.6 at 38
                            # and below (measured; mechanism unknown) — keep
                            # the pipelined mains at 42-44 rows.
TAIL = [10, 7, 5, 3, 3]     # descending tail: each tail row trades 0.324 us
                            # of load time for ~0.21 us of compute, each
                            # extra chunk costs ~0.2 us fixed.  Tail tiles
                            # come from per-chunk bufs=1 pools so no tail
                            # load ever takes a WAR stall (measured
                            # 1.3-1.5 us gaps when tail shared the main pool)
CHUNKS = MAINS + TAIL
IN_SHAPE = (32, 96, 224, 224)
H_OUT = 112

assert sum(CHUNKS) == PAIRS // 128

_cache = {}


def _build():
    import concourse.bass as bass  # noqa: F401
    import concourse.tile as tile
    from concourse import bacc, mybir

    nc = bacc.Bacc("TRN2", target_bir_lowering=False, debug=False)
    x = nc.dram_tensor("x", [PAIRS, 448], mybir.dt.bfloat16, kind="ExternalInput")
    o = nc.dram_tensor("o", [PAIRS, 112], mybir.dt.bfloat16, kind="ExternalOutput")
    xap, oap = x.ap(), o.ap()

    chunks = []
    base = 0
    for mc in CHUNKS:
        chunks.append((base, mc))
        base += 128 * mc

    # Default: loads dedicated to the SP ring, stores to the ACT ring (the
    # best-measured config).  K_SPLIT_LOADS=1 alternates BOTH streams across
    # the two HWDGE rings instead: two queues interleave packets on the 16
    # SDMA engines and fill per-engine HBM-read-latency bubbles (measured
    # 413 GB/s two-queue vs 354 GB/s one-queue on a loads-only phase), but
    # end-to-end both land within noise (~405-413 GB/s aggregate) because
    # stores already fill those slots in dedicated mode.  Stores are EMITTED
    # `LAG` chunks behind their chunk so a sequencer never stalls a ready
    # load behind a waiting-on-compute store (HWDGE rings are FIFO per
    # issuing engine).  SWDGE (gpsimd) stores measured far worse: Q7
    # descriptor generation dribbles ~4 KB packets that eat engine
    # round-robin turns and start ~20 us late.
    LAG = 3
    split = os.environ.get("K_SPLIT_LOADS", "0") == "1"
    rings = None

    def ring(i):
        return rings[i % 2] if split else rings[0]

    def store_ring(k, last):
        if split:
            return rings[(k + 1) % 2]
        # dedicated-ring mode: stores ride ACT, the last one the idle SP ring
        return rings[0] if last else rings[1]

    from contextlib import ExitStack

    with tile.TileContext(nc) as tc, ExitStack() as stack:
        rings = (nc.sync, nc.scalar)
        pin = stack.enter_context(tc.tile_pool(name="inp", bufs=3))
        po = stack.enter_context(tc.tile_pool(name="outp", bufs=4))
        ptails = [
            stack.enter_context(tc.tile_pool(name=f"tail{i}", bufs=1))
            for i in range(len(TAIL))
        ]
        outs = []

        def emit_store(k):
            dst_base, dst_mc = chunks[k]
            dst = oap[dst_base : dst_base + 128 * dst_mc].rearrange(
                "(p m) w -> p (m w)", p=128
            )
            store_ring(k, k == len(chunks) - 1).dma_start(out=dst, in_=outs[k])

        for ci, (base, mc) in enumerate(chunks):
            src = xap[base : base + 128 * mc].rearrange("(p m) w -> p (m w)", p=128)
            pool = pin if ci < len(MAINS) else ptails[ci - len(MAINS)]
            tin = pool.tile([128, mc, 2, 224], mybir.dt.bfloat16)
            ring(ci).dma_start(out=tin[:], in_=src)
            # packed halving 1: [r0e|r1e] vs [r0o|r1o] -> column maxes
            nc.vector.tensor_max(tin[:, :, 0], tin[:, :, 0], tin[:, :, 1])
            to = po.tile([128, mc, 112], mybir.dt.bfloat16)
            # packed halving 2: row-0 col-max vs row-1 col-max
            nc.vector.tensor_max(to[:], tin[:, :, 0, 0:112], tin[:, :, 0, 112:224])
            outs.append(to)
            if ci >= LAG:
                emit_store(ci - LAG)
        for k in range(len(chunks) - LAG, len(chunks)):
            emit_store(k)
    nc.compile()
    return nc


def get_nc():
    if "nc" not in _cache:
        _cache["nc"] = _build()
    return _cache["nc"]


def shard(xp: np.ndarray, c: int) -> dict:
    """xp: full input, bf16, rows already permuted to [r0e|r1e|r0o|r1o]."""
    per_rows = PAIRS
    return {"x": xp[c * per_rows : (c + 1) * per_rows]}


def unshard(outs: list) -> np.ndarray:
    per = IN_SHAPE[0] // N_CORES
    out = np.concatenate(
        [np.asarray(o).reshape(per, IN_SHAPE[1], H_OUT, H_OUT) for o in outs], axis=0
    )
    return out.astype(np.float32)


def make_in_maps(x: np.ndarray) -> list:
    assert x.shape == IN_SHAPE and x.dtype == np.float32, (x.shape, x.dtype)
    xb = np.ascontiguousarray(x).astype(ml_dtypes.bfloat16)
    # (pair, row r, col c, parity j) -> (pair, j, r, c): each 448-elem
    # row-pair becomes [r0_even | r1_even | r0_odd | r1_odd]
    xp = np.ascontiguousarray(
        xb.reshape(-1, 2, 112, 2).transpose(0, 3, 1, 2)
    ).reshape(N_CORES * PAIRS, 448)
    return [shard(xp, c) for c in range(N_CORES)]


def kernel(x: np.ndarray) -> np.ndarray:
    from concourse.bass_utils import run_bass_kernel_spmd

    nc = get_nc()
    in_maps = make_in_maps(x)
    res = run_bass_kernel_spmd(nc, in_maps, list(range(N_CORES)))
    return unshard([res.results[c]["o"] for c in range(N_CORES)])
